# revision 1
# baseline (speedup 1.0000x reference)
"""GQA attention kernel for 8 Trainium2 NeuronCores (Bass/Tile).

Sharding: data-parallel over batch (2) x tensor-parallel over head groups (4).
Core c: batch b=c//4, group g=c%4 (query heads 4g..4g+3, kv head g).
w_q/w_k/w_v column-parallel, w_o row-parallel; partial outputs are
ReduceScattered on-device over groups [[0..3],[4..7]]; host gather is a pure
concatenation.

Hardcoded problem: B=2 T=2048 D=1024 n_heads=16 n_kv=4 d_head=64, causal,
RoPE theta=1e4 (freqs passed as input), scale=1/8.
"""

import numpy as np

import concourse.bass as bass
import concourse.tile as tile
from concourse import bacc, mybir
from concourse.bass_utils import run_bass_kernel_spmd
from concourse.masks import make_identity

F32 = mybir.dt.float32
BF16 = mybir.dt.bfloat16

B, T, D = 2, 2048, 1024
NH, NKV, DH = 16, 4, 64
HPC = NH // NKV          # query heads per core = 4
OC = HPC * DH            # per-core attn feature cols = 256
TB = T // 128            # 16 blocks of 128 rows
NJ = T // 512            # 4 tq-slices of 512
GROUPS = [[0, 1, 2, 3], [4, 5, 6, 7]]
SCALE = 1.0 / 8.0

_CACHE = {}


def _emit(nc, tc, aps):
    x_ap, wq_ap, wk_ap, wv_ap, wo_ap, rope_ap, out_ap = aps
    import contextlib
    ctx = contextlib.ExitStack()
    with ctx:
        sing = ctx.enter_context(tc.tile_pool(name="sing", bufs=1))
        stage = ctx.enter_context(tc.tile_pool(name="stage", bufs=3))
        bstage = ctx.enter_context(tc.tile_pool(name="bstage", bufs=3))
        ropet = ctx.enter_context(tc.tile_pool(name="ropet", bufs=8))
        qrp = ctx.enter_context(tc.tile_pool(name="qrp", bufs=3))
        ptp = ctx.enter_context(tc.tile_pool(name="ptp", bufs=4))
        onatp = ctx.enter_context(tc.tile_pool(name="onatp", bufs=8))
        outsbp = ctx.enter_context(tc.tile_pool(name="outsbp", bufs=3))
        rcp = ctx.enter_context(tc.tile_pool(name="rcp", bufs=8))
        # PSUM pools: trst(3) + qkv(2) + oa(2) + outps(1) = 8 banks
        trstp = ctx.enter_context(tc.tile_pool(name="trstp", bufs=3, space="PSUM"))
        qkvp = ctx.enter_context(tc.tile_pool(name="qkvp", bufs=1, space="PSUM"))
        oap = ctx.enter_context(tc.tile_pool(name="oap", bufs=4, space="PSUM"))
        dram = ctx.enter_context(tc.tile_pool(name="dram", bufs=1, space="DRAM"))

        # ---- warm-up collective: pays the rendezvous cost concurrently with compute
        d_in = dram.tile([1, 64], F32)
        d_out = dram.tile([1, 64], F32)
        zt = sing.tile([1, 64], F32)
        nc.vector.memset(zt[:], 0.0)
        nc.sync.dma_start(d_in[:], zt[:])
        nc.gpsimd.collective_compute(
            "AllReduce", mybir.AluOpType.add, replica_groups=GROUPS,
            ins=[d_in.opt()], outs=[d_out.opt()])

        # ---- persistent SBUF tensors
        identb = sing.tile([128, 128], BF16)
        make_identity(nc, identb[:])
        xT = sing.tile([128, 8, T], BF16)        # [d-chunk part, chunk, t]
        wT = sing.tile([128, 8, 384], BF16)      # cols: 0:256 wq | 256:320 wk | 320:384 wv
        woT = sing.tile([128, 2, D], BF16)       # [o-chunk part, chunk, dout]
        qT = sing.tile([64, 4, T], BF16)         # [d_head part, head, t]
        kT = sing.tile([64, T], BF16)
        vaug = sing.tile([128, TB, 65], BF16)    # col 64 = ones (rowsum trick)
        oT = sing.tile([128, 2, T], BF16)
        rope_sb = sing.tile([128, TB, 5, 64], F32)
        nc.vector.memset(vaug[:], 1.0)
        _r = rope_ap.rearrange("(tb p) f -> p tb f", p=128)
        for _tb in range(TB):
            _rt = _r[:, _tb, :]
            _r5 = bass.AP(tensor=_rt.tensor, offset=_rt.offset,
                          ap=[_rt.ap[0], [0, 5], _rt.ap[1]])
            nc.sync.dma_start(rope_sb[:, _tb, :, :], _r5)

        def cast_transpose(src_ap, n_d, dst_view):
            """src [p,128*n_d] bf16 sbuf -> transposed chunks into dst_view(d)."""
            for dch in range(n_d):
                tr = trstp.tile([128, 512], BF16, tag="trst")
                nc.tensor.transpose(tr[:, :128], src_ap[:, 128 * dch:128 * dch + 128],
                                    identb[:])
                dst, pn = dst_view(dch)
                srcv = tr[:pn, :128] if pn < 128 else tr[:, :128]
                if dch % 3 != 2:
                    nc.vector.tensor_copy(dst, srcv)
                else:
                    nc.scalar.copy(dst, srcv)

        # ---- weights: load, cast to bf16, transpose
        for r in range(2):  # wq rows 256 -> 2 tiles
            wn = stage.tile([128, 1024], F32, tag="wstage")
            nc.sync.dma_start(wn[:], wq_ap[128 * r:128 * (r + 1), :])
            wb = bstage.tile([128, 1024], BF16, tag="wbst")
            nc.vector.tensor_copy(wb[:], wn[:])
            cast_transpose(wb[:], 8, lambda d, r=r: (wT[:, d, 128 * r:128 * (r + 1)], 128))
        for w_ap, col0 in ((wk_ap, 256), (wv_ap, 320)):
            wn = stage.tile([128, 1024], F32, tag="wstage")
            nc.sync.dma_start(wn[:64, :], w_ap[:, :])
            wb = bstage.tile([128, 1024], BF16, tag="wbst")
            nc.vector.tensor_copy(wb[:64, :], wn[:64, :])
            for dch in range(8):
                tr = trstp.tile([128, 512], BF16, tag="trst")
                nc.tensor.transpose(tr[:, :64], wb[:64, 128 * dch:128 * dch + 128],
                                    identb[:64, :64])
                nc.any.tensor_copy(wT[:, dch, col0:col0 + 64], tr[:, :64])
        for r in range(8):  # wo (1024, 256) -> 8 row tiles
            wn = stage.tile([128, 256], F32, tag="wostage")
            nc.sync.dma_start(wn[:], wo_ap[128 * r:128 * (r + 1), :])
            wb = bstage.tile([128, 256], BF16, tag="wobst")
            nc.vector.tensor_copy(wb[:], wn[:])
            cast_transpose(wb[:], 2, lambda oc, r=r: (woT[:, oc, 128 * r:128 * (r + 1)], 128))

        # ---- per t-block: load x, transpose, QKV proj, rope, Q/K transpose
        def phase12(tb):
            xs = stage.tile([128, 1024], F32, tag="xstage")
            nc.sync.dma_start(xs[:], x_ap[128 * tb:128 * (tb + 1), :])
            xb = bstage.tile([128, 1024], BF16, tag="xbst")
            nc.vector.tensor_copy(xb[:], xs[:])
            cast_transpose(xb[:], 8, lambda d, tb=tb: (xT[:, d, 128 * tb:128 * (tb + 1)], 128))
            qkv = qkvp.tile([128, 384], F32, tag="qkv")
            for dch in range(8):
                nc.tensor.matmul(qkv[:], xT[:, dch, 128 * tb:128 * (tb + 1)],
                                 wT[:, dch, :], start=(dch == 0), stop=(dch == 7))
            # V -> vaug (bf16)
            nc.any.tensor_copy(vaug[:, tb, 0:64], qkv[:, 320:384])
            # rope on Q(4 heads)+K(1 head) = 5 groups of 64
            qk = ropet.tile([128, 320], F32, tag="qknat")
            nc.scalar.copy(qk[:], qkv[:, 0:320])
            v4 = qk[:].rearrange("p (g i c) -> p g i c", g=5, c=2)
            re, im = v4[:, :, :, 0], v4[:, :, :, 1]
            rview = rope_sb[:][:, tb, :, :].rearrange("p g (i c) -> p g i c", c=2)
            cos_b, sin_b = rview[:, :, :, 0], rview[:, :, :, 1]
            t1 = ropet.tile([128, 5, 32], F32, tag="t1")
            t2 = ropet.tile([128, 5, 32], F32, tag="t2")
            t3 = ropet.tile([128, 5, 32], F32, tag="t3")
            t4 = ropet.tile([128, 5, 32], F32, tag="t4")
            nc.vector.tensor_mul(t1[:], re, cos_b)
            nc.vector.tensor_mul(t2[:], im, sin_b)
            nc.vector.tensor_mul(t3[:], re, sin_b)
            nc.vector.tensor_mul(t4[:], im, cos_b)
            qr = qrp.tile([128, 320], BF16, tag="qr")
            q4 = qr[:].rearrange("p (g i c) -> p g i c", g=5, c=2)
            nc.vector.tensor_sub(q4[:, :, :, 0], t1[:], t2[:])
            nc.vector.tensor_add(q4[:, :, :, 1], t3[:], t4[:])
            for h in range(4):
                tr = trstp.tile([128, 512], BF16, tag="trst")
                nc.tensor.transpose(tr[:64, :128], qr[:, 64 * h:64 * (h + 1)], identb[:])
                if h % 2 == 0:
                    nc.vector.tensor_copy(qT[:, h, 128 * tb:128 * (tb + 1)], tr[:64, :128])
                else:
                    nc.scalar.copy(qT[:, h, 128 * tb:128 * (tb + 1)], tr[:64, :128])
            tr = trstp.tile([128, 512], BF16, tag="trst")
            nc.tensor.transpose(tr[:64, :128], qr[:, 256:320], identb[:])
            nc.any.tensor_copy(kT[:, 128 * tb:128 * (tb + 1)], tr[:64, :128])

        partial = dram.tile([T, D], F32)

        # ---- attention for tq-slice j (tq 512j..512j+511), all 4 heads
        def phase3(j):
            onats = []
            for c in range(4):
                on = onatp.tile([128, OC], BF16, tag="onat")
                onats.append(on)
            for h in range(4):
                oas = [oap.tile([128, 65], F32, tag="oa", name=f"oa{_c}") for _c in range(4)]
                for i in range(4 * j + 4):
                    o0 = max(0, 128 * i - 512 * j)
                    st = trstp.tile([128, 512], F32, tag="trst")
                    nc.tensor.matmul(
                        st[:, o0:512],
                        kT[:, 128 * i:128 * (i + 1)],
                        qT[:, h, 512 * j + o0:512 * (j + 1)],
                        start=True, stop=True)
                    pt = ptp.tile([128, 512], BF16, tag="pt")
                    nc.scalar.activation(pt[:, o0:512], st[:, o0:512],
                                         mybir.ActivationFunctionType.Exp, scale=SCALE)
                    if i >= 4 * j:  # diagonal block: zero tq < tk after exp
                        c = i - 4 * j
                        nc.gpsimd.affine_select(
                            out=pt[:, 128 * c:128 * (c + 1)],
                            in_=pt[:, 128 * c:128 * (c + 1)],
                            compare_op=mybir.AluOpType.is_ge,
                            fill=0.0, base=0,
                            pattern=[[1, 128]], channel_multiplier=-1)
                    for c in range(4):
                        if i <= 4 * j + c:
                            nc.tensor.matmul(
                                oas[c][:],
                                pt[:, 128 * c:128 * (c + 1)],
                                vaug[:, i, :],
                                start=(i == 0), stop=(i == 4 * j + c))
                for c in range(4):
                    rc = rcp.tile([128, 1], F32, tag="rc")
                    nc.vector.reciprocal(rc[:], oas[c][:, 64:65])
                    nc.vector.tensor_scalar_mul(onats[c][:, DH * h:DH * (h + 1)],
                                                oas[c][:, 0:64], rc[:])
            # O transpose + output projection + partial store, per tq block
            for c in range(4):
                tb = 4 * j + c
                for oc in range(2):
                    tr = trstp.tile([128, 512], BF16, tag="trst")
                    nc.tensor.transpose(tr[:, :128],
                                        onats[c][:, 128 * oc:128 * (oc + 1)], identb[:])
                    if oc == 0:
                        nc.vector.tensor_copy(oT[:, oc, 128 * tb:128 * (tb + 1)], tr[:, :128])
                    else:
                        nc.scalar.copy(oT[:, oc, 128 * tb:128 * (tb + 1)], tr[:, :128])
                for ns in range(2):
                    op = trstp.tile([128, 512], F32, tag="trst", name=f"op{ns}")
                    for oc in range(2):
                        nc.tensor.matmul(op[:], oT[:, oc, 128 * tb:128 * (tb + 1)],
                                         woT[:, oc, 512 * ns:512 * (ns + 1)],
                                         start=(oc == 0), stop=(oc == 1))
                    ob = outsbp.tile([128, 512], F32, tag="outsb")
                    nc.vector.tensor_copy(ob[:], op[:])
                    nc.sync.dma_start(
                        partial[128 * tb:128 * (tb + 1), 512 * ns:512 * (ns + 1)], ob[:])

        for j in range(NJ):
            for tb in range(4 * j, 4 * j + 4):
                phase12(tb)
            phase3(j)
            # rows 512j..512j+512 complete -> ReduceScatter this quarter now
            rsout = dram.tile([128, D], F32, name=f"rsout{j}")
            nc.gpsimd.collective_compute(
                "ReduceScatter", mybir.AluOpType.add, replica_groups=GROUPS,
                ins=[partial[512 * j:512 * (j + 1), :].opt()],
                outs=[rsout.opt()])
            nc.sync.dma_start(out_ap[128 * j:128 * (j + 1), :], rsout[:])


def _build():
    if "nc" in _CACHE:
        return _CACHE["nc"]
    nc = bacc.Bacc("TRN2", target_bir_lowering=False, debug=False, num_devices=8)
    x_ap = nc.dram_tensor("x", [T, D], F32, kind="ExternalInput").ap()
    wq_ap = nc.dram_tensor("wq", [OC, D], F32, kind="ExternalInput").ap()
    wk_ap = nc.dram_tensor("wk", [DH, D], F32, kind="ExternalInput").ap()
    wv_ap = nc.dram_tensor("wv", [DH, D], F32, kind="ExternalInput").ap()
    wo_ap = nc.dram_tensor("wo", [D, OC], F32, kind="ExternalInput").ap()
    rope_ap = nc.dram_tensor("rope", [T, DH], F32, kind="ExternalInput").ap()
    out_ap = nc.dram_tensor("out", [T // 4, D], F32, kind="ExternalOutput").ap()
    with tile.TileContext(nc) as tc:
        _emit(nc, tc, (x_ap, wq_ap, wk_ap, wv_ap, wo_ap, rope_ap, out_ap))
    nc.compile()
    _CACHE["nc"] = nc
    return nc


def run(trace=False, **inputs):
    x = inputs["x"]
    rope2 = np.ascontiguousarray(
        inputs["rope_freqs"].astype(np.float32).reshape(T, DH))
    w_q, w_k, w_v, w_o = (np.asarray(inputs[k], np.float32)
                          for k in ("w_q", "w_k", "w_v", "w_o"))
    nc = _build()
    in_maps = []
    for c in range(8):
        b, g = divmod(c, 4)
        in_maps.append({
            "x": np.ascontiguousarray(x[b], dtype=np.float32),
            "wq": np.ascontiguousarray(w_q[OC * g:OC * (g + 1)]),
            "wk": np.ascontiguousarray(w_k[DH * g:DH * (g + 1)]),
            "wv": np.ascontiguousarray(w_v[DH * g:DH * (g + 1)]),
            "wo": np.ascontiguousarray(w_o[:, OC * g:OC * (g + 1)]),
            "rope": rope2,
        })
    res = run_bass_kernel_spmd(nc, in_maps, core_ids=list(range(8)), trace=trace)
    out = np.empty((B, T, D), np.float32)
    for core in range(8):
        b, r = divmod(core, 4)
        for c in range(4):
            out[b, 512 * c + 128 * r:512 * c + 128 * (r + 1)] = \
                res.results[core]["out"][128 * c:128 * (c + 1)]
    return out, res


def kernel(**inputs):
    out, _ = run(trace=False, **inputs)
    return out



# revision 2
# speedup vs baseline: 1.0145x; 1.0145x over previous
"""GQA attention kernel for 8 Trainium2 NeuronCores (Bass/Tile) — v5.

Sharding: data-parallel over batch (2) x tensor-parallel over head groups (4).
Core c: batch b=c//4, group g=c%4 (query heads 4g..4g+3, kv head g).
w_q/w_k/w_v column-parallel, w_o row-parallel; bf16 partial outputs are
ReduceScattered on-device over groups [[0..3],[4..7]], upcast to fp32 on
device; host gather is a pure concatenation.

Key structure (what each piece is for):
- Q/K are produced directly in transposed [feat, t] layout (weights
  stationary over x^T), with re/im-split feature ordering so RoPE is a few
  wide bf16 vector ops and no per-block Q/K transposes exist.
- Q is stored as head-pair tensors qp[g] = [128, 2, T] whose partition rows
  64:128 are ZERO: score matmuls run K=128 (full PE array keeps the HAM
  clock-gate warm; K=64 tiles run at 1.2 GHz) and one N=512 matmul computes
  a whole head-pair's scores against the shared GQA K block.
- AV likewise: one V-stationary N=512 matmul per head-pair accumulates
  [dv, tq] directly in O-projection lhsT orientation; a ones-column of V
  yields softmax denominators as row 64.
- Denominator reciprocal: the [1,512] rows are transposed to [128,8]
  (8 elems/lane instead of 512 on one lane), reciprocal, transposed back,
  broadcast down 64 partitions with a K=1 matmul, and multiplied in during
  PSUM evacuation.
- The PE queue is in-order, so emission is software-pipelined: AV(i) after
  scores(i+2), per-slice epilogue + output projection deferred into the next
  slice's loop, ReduceScatter finalize deferred a full 512-slice, and the
  next slice's x transposes interleaved into the attention loop as PE
  filler while the scalar engine works through the exps.
"""

import numpy as np

import concourse.bass as bass
import concourse.tile as tile
from concourse import bacc, mybir
from concourse.bass_utils import run_bass_kernel_spmd
from concourse.masks import make_identity

F32 = mybir.dt.float32
BF16 = mybir.dt.bfloat16

B, T, D = 2, 2048, 1024
NH, NKV, DH = 16, 4, 64
HPC = NH // NKV
OC = HPC * DH            # 256
TB = T // 128            # 16
NJ = T // 512            # 4
GROUPS = [[0, 1, 2, 3], [4, 5, 6, 7]]
SCALE = 1.0 / 8.0

_CACHE = {}


def _emit(nc, tc, aps):
    x_ap, wq_ap, wk_ap, wv_ap, wo_ap, rope_ap, out_ap = aps
    import contextlib
    ctx = contextlib.ExitStack()
    with ctx:
        sing = ctx.enter_context(tc.tile_pool(name="sing", bufs=1))
        wstg = ctx.enter_context(tc.tile_pool(name="wstg", bufs=1))
        xsp = ctx.enter_context(tc.tile_pool(name="xsp", bufs=8))
        xbp = ctx.enter_context(tc.tile_pool(name="xbp", bufs=2))
        ropep = ctx.enter_context(tc.tile_pool(name="ropep", bufs=1))
        ktp = ctx.enter_context(tc.tile_pool(name="ktp", bufs=2))
        ptp = ctx.enter_context(tc.tile_pool(name="ptp", bufs=6))
        rcp = ctx.enter_context(tc.tile_pool(name="rcp", bufs=1))
        outsbp = ctx.enter_context(tc.tile_pool(name="outsbp", bufs=2))
        rsfp = ctx.enter_context(tc.tile_pool(name="rsfp", bufs=1))
        pp = ctx.enter_context(tc.tile_pool(name="pp", bufs=1, space="PSUM"))
        dram = ctx.enter_context(tc.tile_pool(name="dram", bufs=1, space="DRAM"))

        def wtile(shape, dtype, name):
            return pp.tile(shape, dtype, tag="st", bufs=6, name=name)

        # ---- warm-up collective: pays the rendezvous cost during setup
        d_in = dram.tile([1, 64], F32)
        d_out = dram.tile([1, 64], F32)
        zt = sing.tile([1, 64], F32)
        nc.vector.memset(zt[:], 0.0)
        nc.sync.dma_start(d_in[:], zt[:])
        nc.gpsimd.collective_compute(
            "AllReduce", mybir.AluOpType.add, replica_groups=GROUPS,
            ins=[d_in.opt()], outs=[d_out.opt()])

        # ---- input staging: few large DMAs
        xs_pref = {}

        def prefetch_x(j):
            if j >= NJ:
                return
            for c in range(4):
                tb = 4 * j + c
                xs = xsp.tile([128, 1024], F32, tag="xs", name=f"xs{tb}")
                nc.sync.dma_start(xs[:], x_ap[128 * tb:128 * (tb + 1), :])
                xs_pref[tb] = xs

        prefetch_x(0)
        ropesb = wstg.tile([128, 16, 64], F32, tag="ropesb")
        nc.sync.dma_start(ropesb[:], rope_ap.rearrange("(tb p) f -> p tb f", p=128))
        wqsb = wstg.tile([128, 2, 1024], F32, tag="wqsb")
        nc.sync.dma_start(wqsb[:], wq_ap.rearrange("(c p) d -> p c d", p=128))
        wksb = wstg.tile([64, 1024], F32, tag="wksb")
        nc.sync.dma_start(wksb[:], wk_ap[:, :])
        wvsb = wstg.tile([64, 1024], F32, tag="wvsb")
        nc.sync.dma_start(wvsb[:], wv_ap[:, :])
        wosb = wstg.tile([128, 8, 256], F32, tag="wosb")
        nc.sync.dma_start(wosb[:], wo_ap.rearrange("(c p) d -> p c d", p=128))

        # ---- persistent SBUF tensors
        identb = sing.tile([128, 128], BF16)
        make_identity(nc, identb[:])
        identf = sing.tile([128, 128], F32)
        make_identity(nc, identf[:])
        Efre = sing.tile([64, 128], F32)
        Efim = sing.tile([64, 128], F32)
        for Ef, p0 in ((Efre, 0), (Efim, 32)):
            nc.vector.memset(Ef[:], 0.0)
            for b4 in range(4):
                nc.vector.tensor_copy(Ef[p0:p0 + 32, 32 * b4:32 * b4 + 32],
                                      identf[p0:p0 + 32, p0:p0 + 32])
        ones1 = sing.tile([1, 64], F32)
        nc.vector.memset(ones1[:], 1.0)

        cos4 = sing.tile([128, T], BF16)   # row 32h+i = cos(t*inv[i])
        sin4 = sing.tile([128, T], BF16)
        xT = sing.tile([128, 8, T], BF16)
        # head-pair Q tensors; rows 64:128 zero (full-K score matmuls)
        qp = [sing.tile([128, 2, T], BF16, name=f"qp{g}") for g in range(2)]
        for g in range(2):
            nc.vector.memset(qp[g][64:128, :, :], 0.0)
        kr64 = sing.tile([128, T], BF16)   # rows 0:64 [kre;kim], 64:128 zero
        nc.vector.memset(kr64[64:128, :], 0.0)
        vaug = sing.tile([128, TB, 65], BF16)
        nc.vector.memset(vaug[:], 1.0)
        wTqre = sing.tile([128, 8, 128], BF16)  # cols: head-major re
        wTqim = sing.tile([128, 8, 128], BF16)
        wTkv = sing.tile([128, 8, 128], BF16)   # cols: kre32 | kim32 | v64
        woT = sing.tile([128, 2, D], BF16)
        oT = sing.tile([128, 2, T], BF16)

        # ---- weights: cast + transpose into lhsT layouts
        wqb = wstg.tile([128, 2, 1024], BF16, tag="wqb")
        nc.vector.tensor_copy(wqb[:], wqsb[:])
        wkvb = wstg.tile([64, 2, 1024], BF16, tag="wkvb")
        nc.vector.tensor_copy(wkvb[:, 0, :], wksb[:])
        nc.vector.tensor_copy(wkvb[:, 1, :], wvsb[:])
        wob = wstg.tile([128, 8, 256], BF16, tag="wob")
        nc.vector.tensor_copy(wob[:], wosb[:])
        for fr in range(2):
            for dch in range(8):
                trw = wtile([128, 128], BF16, name=f"trwq{fr}_{dch}")
                nc.tensor.transpose(trw[:], wqb[:, fr, 128 * dch:128 * (dch + 1)],
                                    identb[:])
                srcv = trw[:].rearrange("p (hp i c) -> p c hp i", hp=2, c=2)
                for hp in range(2):
                    cb = 32 * (2 * fr + hp)
                    nc.vector.tensor_copy(wTqre[:, dch, cb:cb + 32], srcv[:, 0, hp])
                    nc.vector.tensor_copy(wTqim[:, dch, cb:cb + 32], srcv[:, 1, hp])
        for kv in range(2):
            for dch in range(8):
                trw = wtile([128, 128], BF16, name=f"trwk{kv}_{dch}")
                nc.tensor.transpose(trw[:, :64],
                                    wkvb[:, kv, 128 * dch:128 * (dch + 1)],
                                    identb[:64, :64])
                if kv == 1:
                    nc.vector.tensor_copy(wTkv[:, dch, 64:128], trw[:, :64])
                else:
                    srcv = trw[:, :64].rearrange("p (i c) -> p c i", c=2)
                    nc.vector.tensor_copy(wTkv[:, dch, 0:32], srcv[:, 0, :])
                    nc.vector.tensor_copy(wTkv[:, dch, 32:64], srcv[:, 1, :])
        def emit_wo(r):
            for oc in range(2):
                trw = wtile([128, 128], BF16, name=f"trwo{r}_{oc}")
                nc.tensor.transpose(trw[:], wob[:, r, 128 * oc:128 * (oc + 1)],
                                    identb[:])
                nc.vector.tensor_copy(woT[:, oc, 128 * r:128 * (r + 1)], trw[:])

        # ---- rope freqs -> cos4/sin4 (bf16); only chunk 0 on the
        # critical setup path, later chunks become attention-time fillers
        def emit_cs(ch):
            csTc = wstg.tile([64, 512], F32, tag="csTc", bufs=2)
            for t4 in range(4):
                tb = 4 * ch + t4
                css = wstg.tile([128, 64], F32, tag="csstage", bufs=2)
                nc.vector.tensor_copy(
                    css[:].rearrange("p (c i) -> p c i", c=2),
                    ropesb[:, tb, :].rearrange("p (i c) -> p c i", c=2))
                trc = wtile([64, 128], F32, name=f"trc{tb}")
                nc.tensor.transpose(trc[:], css[:], identf[:])
                nc.vector.tensor_copy(csTc[:, 128 * t4:128 * (t4 + 1)], trc[:])
            sl = slice(512 * ch, 512 * (ch + 1))
            cps = wtile([128, 512], F32, name=f"cps{ch}")
            nc.tensor.matmul(cps[:], Efre[:], csTc[:], start=True, stop=True)
            nc.vector.tensor_copy(cos4[:, sl], cps[:])
            sps = wtile([128, 512], F32, name=f"sps{ch}")
            nc.tensor.matmul(sps[:], Efim[:], csTc[:], start=True, stop=True)
            nc.vector.tensor_copy(sin4[:, sl], sps[:])

        emit_cs(0)

        partial = dram.tile([T, D], BF16)

        # ---- x transpose for one 128-row block (PE filler, interleavable)
        def emit_trx(tb):
            xs = xs_pref.pop(tb)
            xb = xbp.tile([128, 1024], BF16, tag="xb")
            nc.vector.tensor_copy(xb[:], xs[:])
            trx = wtile([128, 1024], BF16, name=f"trx{tb}")
            for d8 in range(8):
                nc.tensor.transpose(trx[:, 128 * d8:128 * (d8 + 1)],
                                    xb[:, 128 * d8:128 * (d8 + 1)], identb[:])
            nc.vector.tensor_copy(
                xT[:, :, 128 * tb:128 * (tb + 1)],
                trx[:].rearrange("p (d t) -> p d t", d=8))

        # ---- QKV projection + rope for a 512-slice (xT must be ready),
        # split into two emitters so they can be injected into the previous
        # slice's attention loop as PE filler.
        def emit_kv(j):
            jsl = slice(512 * j, 512 * (j + 1))
            kv_ps = wtile([128, 512], F32, name=f"kv{j}")
            for dch in range(8):
                nc.tensor.matmul(kv_ps[:], wTkv[:, dch, :], xT[:, dch, jsl],
                                 start=(dch == 0), stop=(dch == 7))
            kvre = ktp.tile([32, 512], BF16, tag="kvre")
            nc.vector.tensor_copy(kvre[:], kv_ps[0:32, :])
            kvim = ktp.tile([32, 512], BF16, tag="kvim")
            nc.vector.tensor_copy(kvim[:], kv_ps[32:64, :])
            vtmp = ktp.tile([64, 512], BF16, tag="vtmp")
            nc.scalar.copy(vtmp[:], kv_ps[64:128, :])
            ktmp = ktp.tile([64, 512], BF16, tag="ktmp")
            k1 = ropep.tile([32, 512], BF16, tag="k1")
            k2 = ropep.tile([32, 512], BF16, tag="k2")
            nc.vector.tensor_mul(k1[:], kvre[:], cos4[0:32, jsl])
            nc.vector.tensor_mul(k2[:], kvim[:], sin4[0:32, jsl])
            nc.vector.tensor_sub(ktmp[0:32, :], k1[:], k2[:])
            k3 = ropep.tile([32, 512], BF16, tag="k3")
            k4 = ropep.tile([32, 512], BF16, tag="k4")
            nc.vector.tensor_mul(k3[:], kvre[:], sin4[0:32, jsl])
            nc.vector.tensor_mul(k4[:], kvim[:], cos4[0:32, jsl])
            nc.vector.tensor_add(ktmp[32:64, :], k3[:], k4[:])
            nc.vector.tensor_copy(kr64[0:64, jsl], ktmp[:])
            for c in range(4):
                trv = wtile([128, 64], BF16, name=f"trv{4*j+c}")
                nc.tensor.transpose(trv[:], vtmp[:, 128 * c:128 * (c + 1)],
                                    identb[:64, :64])
                nc.vector.tensor_copy(vaug[:, 4 * j + c, 0:64], trv[:])

        def emit_q(j):
            jsl = slice(512 * j, 512 * (j + 1))
            # Q blocks; rope in bf16 on the vector engine
            qre_ps = wtile([128, 512], F32, name=f"qre{j}")
            for dch in range(8):
                nc.tensor.matmul(qre_ps[:], wTqre[:, dch, :], xT[:, dch, jsl],
                                 start=(dch == 0), stop=(dch == 7))
            qim_ps = wtile([128, 512], F32, name=f"qim{j}")
            for dch in range(8):
                nc.tensor.matmul(qim_ps[:], wTqim[:, dch, :], xT[:, dch, jsl],
                                 start=(dch == 0), stop=(dch == 7))
            qreb = ktp.tile([128, 512], BF16, tag="qreb")
            nc.scalar.copy(qreb[:], qre_ps[:])
            qimb = ktp.tile([128, 512], BF16, tag="qimb")
            nc.scalar.copy(qimb[:], qim_ps[:])
            t1 = ropep.tile([128, 512], BF16, tag="t1")
            t2 = ropep.tile([128, 512], BF16, tag="t2")
            nc.vector.tensor_mul(t1[:], qreb[:], cos4[:, jsl])
            nc.vector.tensor_mul(t2[:], qimb[:], sin4[:, jsl])
            t3 = ropep.tile([128, 512], BF16, tag="t3")
            t4 = ropep.tile([128, 512], BF16, tag="t4")
            nc.vector.tensor_mul(t3[:], qreb[:], sin4[:, jsl])
            nc.vector.tensor_mul(t4[:], qimb[:], cos4[:, jsl])
            for h in range(4):
                s0 = 32 * h
                dst = qp[h // 2]
                nc.vector.tensor_sub(dst[0:32, h % 2, jsl], t1[s0:s0 + 32, :],
                                     t2[s0:s0 + 32, :])
                nc.vector.tensor_add(dst[32:64, h % 2, jsl], t3[s0:s0 + 32, :],
                                     t4[s0:s0 + 32, :])

        # ---- attention: one N=512 score MM and one N=512 AV MM per
        # head-pair per tk-block; AV skewed 2 iterations behind scores.
        def att_scores(jp, i, state):
            tqsl = slice(256 * jp, 256 * (jp + 1))
            ksl = slice(128 * i, 128 * (i + 1))
            pts = []
            for g in range(2):
                st = pp.tile([128, 512], F32, tag="st", bufs=6,
                             name=f"st{jp}_{i}_{g}")
                nc.tensor.matmul(st[:], kr64[:, ksl], qp[g][:, :, tqsl],
                                 start=True, stop=True)
                pt = ptp.tile([128, 512], BF16, tag="pt", name=f"pt{jp}_{i}_{g}")
                nc.scalar.activation(pt[:], st[:],
                                     mybir.ActivationFunctionType.Exp,
                                     scale=SCALE)
                pts.append(pt)
            if i >= 2 * jp:  # diagonal blocks: zero tq < tk after exp
                base = 256 * jp - 128 * i
                for pt in pts:
                    nc.gpsimd.affine_select(
                        out=pt[:].rearrange("p (a b) -> p a b", a=2),
                        in_=pt[:].rearrange("p (a b) -> p a b", a=2),
                        compare_op=mybir.AluOpType.is_ge,
                        fill=0.0, base=base,
                        pattern=[[0, 2], [1, 256]], channel_multiplier=-1)
            state[i] = pts

        def att_av(jp, i, state, avs, ilast):
            pts = state.pop(i)
            for g in range(2):
                nc.tensor.matmul(avs[g][:], vaug[:, i, :], pts[g][:],
                                 start=(i == 0), stop=(i == ilast))

        def att_epilogue(jp, avs):
            avX, avY = avs
            dr = rcp.tile([1, 1024], F32, tag="dr")
            nc.vector.tensor_copy(dr[:, 0:512], avX[64:65, :])
            nc.vector.tensor_copy(dr[:, 512:1024], avY[64:65, :])
            dnT = wtile([128, 8], F32, name=f"dnT{jp}")
            for c in range(8):
                nc.tensor.transpose(dnT[:, c:c + 1], dr[:, 128 * c:128 * (c + 1)],
                                    identf[0:1, 0:1])
            rcT = rcp.tile([128, 8], F32, tag="rcT")
            nc.vector.reciprocal(rcT[:], dnT[:])
            for bi, av in enumerate((avX, avY)):
                rcr = wtile([1, 512], F32, name=f"rcr{jp}_{bi}")
                for c in range(4):
                    nc.tensor.transpose(rcr[0:1, 128 * c:128 * (c + 1)],
                                        rcT[:, 4 * bi + c:4 * bi + c + 1],
                                        identf[:])
                rcs = rcp.tile([1, 512], F32, tag="rcs")
                nc.vector.tensor_copy(rcs[:], rcr[:])
                bc = wtile([64, 512], F32, name=f"bc{jp}_{bi}")
                nc.tensor.matmul(bc[:], ones1[:], rcs[:], start=True, stop=True)
                bcs = rcp.tile([64, 512], F32, tag="bcs")
                nc.scalar.copy(bcs[:], bc[:])
                for hh in range(2):
                    csl = slice(256 * hh, 256 * (hh + 1))
                    nc.vector.tensor_mul(oT[64 * hh:64 * (hh + 1), bi,
                                            256 * jp:256 * (jp + 1)],
                                         av[0:64, csl], bcs[0:64, csl])

        def phase_oproj(tb):
            for ns in range(2):
                op = wtile([128, 512], F32, name=f"op{tb}_{ns}")
                for oc in range(2):
                    nc.tensor.matmul(op[:], oT[:, oc, 128 * tb:128 * (tb + 1)],
                                     woT[:, oc, 512 * ns:512 * (ns + 1)],
                                     start=(oc == 0), stop=(oc == 1))
                ob = outsbp.tile([128, 512], BF16, tag="outsb")
                nc.vector.tensor_copy(ob[:], op[:])
                nc.sync.dma_start(
                    partial[128 * tb:128 * (tb + 1), 512 * ns:512 * (ns + 1)], ob[:])

        # ---- main loop with cross-slice software pipelining
        pending = []   # epilogue/O-proj emitters, drained next slice
        fins = []      # RS finalizers, drained a full 512-slice later so the
                       # vector-queue cast never waits on the collective
        av_live = {}

        def drain_one():
            if pending:
                pending.pop(0)()

        def drain_pending():
            while pending:
                pending.pop(0)()

        def drain_fins():
            while fins:
                fins.pop(0)()

        # ---- setup: slice 0 inline, wo-prep as early filler
        for tb in range(4):
            emit_trx(tb)
        emit_kv(0)
        emit_q(0)
        fillers = [(lambda r=r: emit_wo(r)) for r in range(8)]

        def drain_filler():
            if fillers:
                fillers.pop(0)()

        for j in range(NJ):
            if j == 0:
                while fillers:
                    drain_filler()   # wo prep; overlaps att(0)'s exps
            else:
                emit_kv(j)
                emit_q(j)
            prefetch_x(j + 1)
            # x transposes for j+1, doled out as PE filler during attention
            trx_todo = list(range(4 * j + 4, min(4 * j + 8, TB)))
            if trx_todo:
                emit_trx(trx_todo.pop(0))  # hides the rope->kr64 chain
            for jp in (2 * j, 2 * j + 1):
                ilast = 2 * jp + 1
                avX = pp.tile([65, 512], F32, tag="av", bufs=2, name=f"avX{jp}")
                avY = pp.tile([65, 512], F32, tag="av", bufs=2, name=f"avY{jp}")
                avs = (avX, avY)
                av_live[jp] = avs
                state = {}
                for i in range(ilast + 1):
                    att_scores(jp, i, state)
                    if i == 2 or (i == 1 and ilast < 2):
                        drain_one()
                    if i == 3 and jp == 2 * j + 1:
                        drain_fins()   # RS(j-1) long done by now
                    if i >= 2:
                        att_av(jp, i - 2, state, avs, ilast)
                    if i % 3 == 1 and trx_todo:
                        emit_trx(trx_todo.pop(0))
                for i in (max(ilast - 1, 0), ilast):
                    if i in state:
                        att_av(jp, i, state, avs, ilast)

                def mk(jp=jp):
                    a = av_live.pop(jp)

                    def emit():
                        att_epilogue(jp, a)
                        phase_oproj(2 * jp)
                        phase_oproj(2 * jp + 1)
                    return emit
                pending.append(mk())
            while trx_todo:
                emit_trx(trx_todo.pop(0))
            if j + 1 < NJ:
                emit_cs(j + 1)
            drain_pending()
            rsout = dram.tile([128, D], BF16, name=f"rsout{j}")
            nc.gpsimd.collective_compute(
                "ReduceScatter", mybir.AluOpType.add, replica_groups=GROUPS,
                ins=[partial[512 * j:512 * (j + 1), :].opt()],
                outs=[rsout.opt()])

            def mkfin(j=j, rsout=rsout):
                def emit():
                    rsb = rsfp.tile([128, D], BF16, tag="rsb")
                    nc.sync.dma_start(rsb[:], rsout[:])
                    rsf = rsfp.tile([128, D], F32, tag="rsf")
                    nc.vector.tensor_copy(rsf[:], rsb[:])
                    nc.sync.dma_start(out_ap[128 * j:128 * (j + 1), :], rsf[:])
                return emit
            fins.append(mkfin())
        drain_fins()


def _build():
    if "nc" in _CACHE:
        return _CACHE["nc"]
    nc = bacc.Bacc("TRN2", target_bir_lowering=False, debug=False, num_devices=8)
    x_ap = nc.dram_tensor("x", [T, D], F32, kind="ExternalInput").ap()
    wq_ap = nc.dram_tensor("wq", [OC, D], F32, kind="ExternalInput").ap()
    wk_ap = nc.dram_tensor("wk", [DH, D], F32, kind="ExternalInput").ap()
    wv_ap = nc.dram_tensor("wv", [DH, D], F32, kind="ExternalInput").ap()
    wo_ap = nc.dram_tensor("wo", [D, OC], F32, kind="ExternalInput").ap()
    rope_ap = nc.dram_tensor("rope", [T, DH], F32, kind="ExternalInput").ap()
    out_ap = nc.dram_tensor("out", [T // 4, D], F32, kind="ExternalOutput").ap()
    with tile.TileContext(nc) as tc:
        _emit(nc, tc, (x_ap, wq_ap, wk_ap, wv_ap, wo_ap, rope_ap, out_ap))
    nc.compile()
    _CACHE["nc"] = nc
    return nc


def run(trace=False, **inputs):
    x = inputs["x"]
    rope2 = np.ascontiguousarray(
        inputs["rope_freqs"].astype(np.float32).reshape(T, DH))
    w_q, w_k, w_v, w_o = (np.asarray(inputs[k], np.float32)
                          for k in ("w_q", "w_k", "w_v", "w_o"))
    nc = _build()
    in_maps = []
    for c in range(8):
        b, g = divmod(c, 4)
        in_maps.append({
            "x": np.ascontiguousarray(x[b], dtype=np.float32),
            "wq": np.ascontiguousarray(w_q[OC * g:OC * (g + 1)]),
            "wk": np.ascontiguousarray(w_k[DH * g:DH * (g + 1)]),
            "wv": np.ascontiguousarray(w_v[DH * g:DH * (g + 1)]),
            "wo": np.ascontiguousarray(w_o[:, OC * g:OC * (g + 1)]),
            "rope": rope2,
        })
    res = run_bass_kernel_spmd(nc, in_maps, core_ids=list(range(8)), trace=trace)
    out = np.empty((B, T, D), np.float32)
    for core in range(8):
        b, r = divmod(core, 4)
        for c in range(4):
            out[b, 512 * c + 128 * r:512 * c + 128 * (r + 1)] = \
                res.results[core]["out"][128 * c:128 * (c + 1)]
    return out, res


def kernel(**inputs):
    out, _ = run(trace=False, **inputs)
    return out
